# revision 10
# baseline (speedup 1.0000x reference)
"""ConvNearestNeightbor Trainium2 kernel (v2).

out[b, n*C+c, i, j] = max_k |x[b,c,i-r_k,j-c_k] - neighbors[n,c,k]|
over the 9 zero-padded 3x3 shifts (r_k, c_k).

Sharding: 8 cores = 4 batch-groups x 2 num-groups.
Per core: B_loc=4 batches, N_loc=16 codebook entries.
Partition layout: (nn in 0..3, c in 0..31) -> 128 partitions; chain nt
in 0..3 selects n = nt*4+nn. Each chain processes the full local batch
(free = 4*32*32 = 4096).

Per chain: 9 signed/abs planes are produced -- ACT (Abs activation,
1x rate, takes the 4B-misaligned window shifts {1,4,7} plus filler)
and DVE (tensor_scalar subtract at 4x into a grouped tile, one
bitwise-and sign-clear at 4x over the group). DVE folds everything
with tensor_tensor max at 2x: eager pair tree over all planes except
the last ACT plane, then a depth-1 final fold per batch-half, each
half DMAed out immediately (SWDGE fp16->f32 cast).

Emission is software-pipelined: chain i's fold tree is emitted after
chain i+1's produce ops so the in-order DVE queue never stalls on ACT.
"""

import numpy as np

B, C, H, W = 16, 32, 32, 32
NUM = 32
NCORES = 8
BG, NG = 4, 2          # batch groups x num groups
B_LOC = B // BG        # 4
N_LOC = NUM // NG      # 16
NT = N_LOC // 4        # 4 chains of 4 n each
PH, PW = H + 2, W + 2  # 34 x 34 padded image
FREE = B_LOC * H * W   # 4096
HALF = FREE // 2       # 2048

# Per-chain DVE-produced ks. Must be 4B-aligned window offsets
# (a*34+b even): legal set {0,2,3,5,6,8}. ACT takes the rest; ACT's
# emission order puts k=4 first (reads unpadded xraw, so the chain
# starts before the pad copy) and the remaining filler last.
CHAIN_DVE_KS = [(6, 8), (6, 8), (2, 6, 8), (2, 6, 8)]

_module_cache = {}


def _build_module():
    import concourse.bacc as bacc
    import concourse.mybir as mybir
    import concourse.tile as tile

    dt = mybir.dt
    Alu = mybir.AluOpType
    AF = mybir.ActivationFunctionType

    nc = bacc.Bacc("TRN2", debug=False)
    x = nc.dram_tensor("x", [B_LOC, C, H, W], dt.float32, kind="ExternalInput")
    nb = nc.dram_tensor("neighbors", [N_LOC, C, 9], dt.float32, kind="ExternalInput")
    out = nc.dram_tensor(
        "out", [B_LOC, N_LOC * C, H, W], dt.float32, kind="ExternalOutput"
    )

    # window start offsets within the padded 34x34 image for the 9 shifts
    offs = []
    for row in (-1, 0, 1):
        for col in (-1, 0, 1):
            offs.append((1 - row, 1 - col))

    with tile.TileContext(nc) as tc:
        with (
            tc.tile_pool(name="const", bufs=1) as cpool,
            tc.tile_pool(name="dp", bufs=11) as dpool,
            tc.tile_pool(name="dvp", bufs=2) as dvpool,
        ):
            # border zeros for the padded image; gpsimd is idle at t=0
            xpad = cpool.tile([128, B_LOC * PH * PW], dt.float16, tag="xpad")
            nc.gpsimd.memset(xpad[:], 0.0)
            xpad_v = xpad[:].rearrange("p (b h w) -> p b h w", b=B_LOC, h=PH, w=PW)

            # neighbors first on sync: tiny DMA gating nbneg -> first ABS bias
            nbt = cpool.tile([128, NT * 9], dt.float32, tag="nbt")
            nb_src = nb.ap().rearrange("(t nn) c k -> (nn c) t k", nn=4)
            nbt_v = nbt[:].rearrange("p (t k) -> p t k", t=NT)
            nc.sync.dma_start(nbt_v, nb_src)

            # x replicated into all 4 nn partition groups, h0 halves first,
            # one HWDGE queue per replica so h0 lands as early as possible
            x_src = x.ap().rearrange("b c h w -> c b (h w)")
            xraw = cpool.tile([128, FREE], dt.float32, tag="xraw")
            xraw_v = xraw[:].rearrange("p (b s) -> p b s", b=B_LOC)
            engs = [nc.sync, nc.scalar, nc.sync, nc.scalar]
            for h in range(2):
                for nn in range(4):
                    engs[nn].dma_start(
                        xraw_v[nn * 32 : (nn + 1) * 32, 2 * h : 2 * h + 2],
                        x_src[:, 2 * h : 2 * h + 2],
                    )

            # negated neighbors: ACT bias computes Abs(x + (-nb))
            nbneg = cpool.tile([128, NT * 9], dt.float32, tag="nbneg")
            nc.scalar.mul(nbneg[:], nbt[:], -1.0)

            # pad interior per half on DVE (fp32 -> fp16 cast copy, 2x)
            for h in range(2):
                nc.vector.tensor_scalar(
                    xpad_v[:, 2 * h : 2 * h + 2, 1 : 1 + H, 1 : 1 + W],
                    xraw_v[:, 2 * h : 2 * h + 2].rearrange(
                        "p b (h w) -> p b h w", h=H
                    ),
                    1.0, None, Alu.mult,
                )

            out_v = out.ap().rearrange("b (t p) h w -> t p b (h w)", t=NT)

            def emit_produce(ci, nt):
                dve_ks = CHAIN_DVE_KS[ci]
                act_ks = [4, 1, 7] + [
                    k for k in (0, 2, 3, 5) if k not in dve_ks
                ]
                # ACT planes
                planes = {}
                for ki, k in enumerate(act_ks):
                    d = dpool.tile([128, FREE], dt.float16, tag="d")
                    d_v = d[:].rearrange("p (b h w) -> p b h w", b=B_LOC, h=H, w=W)
                    bias = nbneg[:, nt * 9 + k : nt * 9 + k + 1]
                    split = (ci == 0 and k == 4) or (
                        ci == len(CHAIN_DVE_KS) - 1 and ki == len(act_ks) - 1
                    )
                    if k == 4:
                        srcs = [xraw_v]
                    else:
                        a, bc = offs[k]
                        srcs = [xpad_v[:, :, a : a + H, bc : bc + W]]
                    if split:
                        for h in range(2):
                            nc.scalar.activation(
                                d_v[:, 2 * h : 2 * h + 2],
                                srcs[0][:, 2 * h : 2 * h + 2],
                                AF.Abs, bias=bias, scale=1.0,
                            )
                    else:
                        nc.scalar.activation(
                            d_v, srcs[0], AF.Abs, bias=bias, scale=1.0
                        )
                    planes[k] = d
                # DVE planes into one grouped tile + one mask + v-fold
                v = len(dve_ks)
                dv = dvpool.tile([128, v * FREE], dt.float16, tag="dv")
                dv_g = dv[:].rearrange("p (v b h w) -> p v b h w", v=v, b=B_LOC, h=H)
                for i, k in enumerate(dve_ks):
                    a, bc = offs[k]
                    nc.vector.tensor_scalar(
                        dv_g[:, i],
                        xpad_v[:, :, a : a + H, bc : bc + W],
                        nbt[:, nt * 9 + k : nt * 9 + k + 1],
                        None, Alu.subtract,
                    )
                nc.vector.tensor_scalar(
                    dv[:].bitcast(dt.uint16), dv[:].bitcast(dt.uint16),
                    0x7FFF, None, Alu.bitwise_and,
                )
                dv_p = dv[:].rearrange("p (v s) -> p v s", v=v)
                # fold DVE planes into dv slice 0
                for i in range(1, v):
                    nc.vector.tensor_tensor(
                        dv_p[:, 0], dv_p[:, 0], dv_p[:, i], Alu.max
                    )
                return planes, act_ks, dv_p

            def emit_folds(nt, planes, act_ks, dv_p, tail):
                # serial chain in plane-arrival order (folds become ready
                # progressively, minimizing post-ACT exposure), vacc last,
                # then a depth-1 final fold per slice with immediate DMA
                acc = planes[act_ks[0]][:]
                for k in act_ks[1:-1]:
                    nc.vector.tensor_tensor(acc, acc, planes[k][:], Alu.max)
                nc.vector.tensor_tensor(acc, acc, dv_p[:, 0], Alu.max)
                last = planes[act_ks[-1]]
                last_s = last[:].rearrange("p (b s) -> p b s", b=B_LOC)
                parts = 4 if tail else 2
                step = FREE // parts
                bstep = B_LOC // parts
                for q in range(parts):
                    sl = slice(q * step, (q + 1) * step)
                    nc.vector.tensor_tensor(
                        last[:, sl], last[:, sl], acc[:, sl], Alu.max
                    )
                    nc.gpsimd.dma_start(
                        out_v[nt][:, q * bstep : (q + 1) * bstep],
                        last_s[:, q * bstep : (q + 1) * bstep],
                    )

            pending = None
            for ci in range(NT):
                state = emit_produce(ci, ci)
                if pending is not None:
                    emit_folds(pending[0], *pending[1], tail=False)
                pending = (ci, state)
            emit_folds(pending[0], *pending[1], tail=True)

    nc.compile()
    return nc


def _get_module():
    if "nc" not in _module_cache:
        _module_cache["nc"] = _build_module()
    return _module_cache["nc"]


def _run(x, neighbors, trace=False):
    from concourse import bass_utils

    x = np.ascontiguousarray(x, dtype=np.float32)
    neighbors = np.ascontiguousarray(neighbors, dtype=np.float32)
    in_maps = []
    for core in range(NCORES):
        bg, ng = divmod(core, NG)
        in_maps.append(
            {
                "x": x[bg * B_LOC : (bg + 1) * B_LOC],
                "neighbors": neighbors[ng * N_LOC : (ng + 1) * N_LOC],
            }
        )
    res = bass_utils.run_bass_kernel_spmd(
        _get_module(), in_maps, core_ids=list(range(NCORES)), trace=trace
    )
    out = np.empty((B, NUM * C, H, W), dtype=np.float32)
    for core in range(NCORES):
        bg, ng = divmod(core, NG)
        out[bg * B_LOC : (bg + 1) * B_LOC, ng * N_LOC * C : (ng + 1) * N_LOC * C] = (
            res.results[core]["out"]
        )
    return out, res


def kernel(x, neighbors):
    out, _ = _run(x, neighbors, trace=False)
    return out
